# revision 10
# baseline (speedup 1.0000x reference)
"""MoE FFN (D=1024, F=4096, E=4, top-2) Trainium2 Bass kernel.

Strategy: expert-parallel dispatch. The router (a 8192x1024x4 matmul +
softmax + top-2) is computed on host in float64 -- it is 0.01% of the
model FLOPs and its only role is to pick the token->expert assignment
that defines the sharding.  Each expert is served by 2 of the 8 cores;
the host gathers each core's assigned tokens into a transposed
[D, C] activation block (capacity C, zero-padded), so the device kernel
is a dense single-expert FFN with tokens on the free dimension:

  h[F, C]  = gelu(W1^T @ xT + b1)        (bf16 matmuls, ACT gelu)
  oT[D, C] = g * (W2^T @ h + b2)         (bf16 matmuls, fp32 psum/acc)

The host then scatter-adds the two gated expert outputs per token.
Only the 2 selected experts per token are ever computed (2x fewer
matmul FLOPs than the dense reference), weights are read from HBM
exactly once, and there are no on-device transposes.

bf16 runs the PE at 1 cycle/row (same as fp32r) but halves LDWEIGHTS
time (stationary reload is the per-matmul overhead) and all weight DMA.
PSUM accumulation stays fp32; only matmul operand storage is bf16.

DMA queues are specialized so nothing blocks the critical path:
gpsimd = biases/gates + weight slabs, sync = xT + output, and the
scalar engine issues no DMA (gelu drains psum on the critical path).
"""
import numpy as np
import ml_dtypes
from contextlib import ExitStack

import concourse.bass as bass
import concourse.tile as tile
from concourse import mybir, bacc
from concourse.bass_utils import run_bass_kernel_spmd

DT = mybir.dt
AFT = mybir.ActivationFunctionType
ALU = mybir.AluOpType

N_CORES = 8
B, S, D, F, E = 4, 2048, 1024, 4096, 4
TOKENS = B * S                  # 8192 tokens, 16384 (token, expert) jobs
TOP_K = 2
P = 128
DC = D // P                     # 8 d-chunks
FC = F // P                     # 32 f-chunks
NCC = 5                         # token chunks per core (each <= 512 psum cols)
C_DEFAULT = 2120                # per-core job capacity (seed-0 max load: 2101)
NPH = 4                         # F phases; FPH f-chunks of h live at a time
FPH = FC // NPH                 # 8
MM_DT = DT.bfloat16
NP_MM = ml_dtypes.bfloat16

_CACHE = {}


def _ffn(tc, xT, w1, b1, w2, b2, g, out, C):
    nc = tc.nc
    CC = C // NCC
    FW = FPH * P                # 1024 f columns per W1 phase slab
    with ExitStack() as ctx:
        singles = ctx.enter_context(tc.tile_pool(name="singles", bufs=1))
        b1_sb = singles.tile([P, FC], DT.float32)
        b2_sb = singles.tile([P, DC], DT.float32)
        G = singles.tile([P, C], DT.float32)

        # resident activations: xT (input, bf16) and acc (fp32 partials)
        xp = ctx.enter_context(tc.tile_pool(name="xp", bufs=1))
        xts = [[xp.tile([P, CC], MM_DT, name=f"x{d}_{c}") for c in range(NCC)]
               for d in range(DC)]
        accp = ctx.enter_context(tc.tile_pool(name="acc", bufs=1))
        acc = [accp.tile([P, C], DT.float32, name=f"acc{d}") for d in range(DC)]

        # constants on the scalar queue (idle until the first gelu, which
        # needs b1 anyway); keeps gpsimd free for the phase-0 weight slabs
        nc.scalar.dma_start(b1_sb[:], b1.rearrange("(f p) -> p f", p=P))
        nc.scalar.dma_start(b2_sb[:], b2.rearrange("(d p) -> p d", p=P))
        nc.scalar.dma_start(G[:], bass.AP(tensor=g.tensor, offset=0,
                                          ap=[[0, P], [1, C]]))
        for c in range(NCC):
            for d in range(DC):
                nc.sync.dma_start(xts[d][c][:],
                                  xT[d * P:(d + 1) * P, c * CC:(c + 1) * CC])

        hp = ctx.enter_context(tc.tile_pool(name="hp", bufs=1))
        h = [hp.tile([P, C], MM_DT, name=f"h{f}") for f in range(FPH)]
        # batched weight slabs: W1 [128d x 1024f] per (phase, d);
        # W2 [128f x 1024d(=D)] per f-chunk.  One contiguous DMA each.
        w1p = ctx.enter_context(tc.tile_pool(name="w1p", bufs=10))
        w2p = ctx.enter_context(tc.tile_pool(name="w2p", bufs=10))
        ps1 = ctx.enter_context(tc.tile_pool(name="ps1", bufs=4, space="PSUM"))
        ps2 = ctx.enter_context(tc.tile_pool(name="ps2", bufs=4, space="PSUM"))
        op = ctx.enter_context(tc.tile_pool(name="op", bufs=8))

        for ph in range(NPH):
            f0 = ph * FPH
            # ---- W1 slab: h[fi] = gelu(W1[:, slab]^T xT + b1) ----
            w1t = [w1p.tile([P, FW], MM_DT, name="w1t") for _ in range(DC)]
            for d in range(DC):
                nc.gpsimd.dma_start(w1t[d][:],
                                    w1[d * P:(d + 1) * P, f0 * P:f0 * P + FW])
            w2t = [w2p.tile([P, D], MM_DT, name="w2t") for _ in range(FPH)]
            for fi in range(FPH):
                fg = f0 + fi
                nc.gpsimd.dma_start(w2t[fi][:], w2[fg * P:(fg + 1) * P, :])
            for c in range(NCC):
                for fi in range(FPH):
                    pt = ps1.tile([P, CC], DT.float32, name="pt")
                    for d in range(DC):
                        nc.tensor.matmul(pt[:],
                                         w1t[d][:, fi * P:(fi + 1) * P],
                                         xts[d][c][:],
                                         start=(d == 0), stop=(d == DC - 1))
                    nc.scalar.activation(h[fi][:, c * CC:(c + 1) * CC], pt[:],
                                         AFT.Gelu, bias=b1_sb[:, f0 + fi:f0 + fi + 1],
                                         scale=1.0)
            # ---- W2 slab: acc[d] += W2[slab, :]^T h ----
            for d in range(DC):
                for c in range(NCC):
                    pt = ps2.tile([P, CC], DT.float32, name="pt2")
                    for fi in range(FPH):
                        nc.tensor.matmul(pt[:],
                                         w2t[fi][:, d * P:(d + 1) * P],
                                         h[fi][:, c * CC:(c + 1) * CC],
                                         start=(fi == 0), stop=(fi == FPH - 1))
                    csl = slice(c * CC, (c + 1) * CC)
                    if ph == 0:
                        # seed acc with b2 while copying out of psum
                        nc.scalar.activation(acc[d][:, csl], pt[:], AFT.Identity,
                                             bias=b2_sb[:, d:d + 1], scale=1.0)
                    elif ph < NPH - 1:
                        nc.vector.tensor_add(acc[d][:, csl], acc[d][:, csl], pt[:])
                    else:
                        t = op.tile([P, CC], DT.float32, name="ot")
                        nc.vector.tensor_add(t[:], acc[d][:, csl], pt[:])
                        nc.vector.tensor_mul(t[:], t[:], G[:, csl])
                        nc.sync.dma_start(out[d * P:(d + 1) * P, csl], t[:])


def _build(C):
    nc = bacc.Bacc("TRN2", target_bir_lowering=False, debug=False,
                   num_devices=N_CORES)
    xT = nc.dram_tensor("xt", [D, C], MM_DT, kind="ExternalInput").ap()
    w1 = nc.dram_tensor("w1", [D, F], MM_DT, kind="ExternalInput").ap()
    b1 = nc.dram_tensor("b1", [F], DT.float32, kind="ExternalInput").ap()
    w2 = nc.dram_tensor("w2", [F, D], MM_DT, kind="ExternalInput").ap()
    b2 = nc.dram_tensor("b2", [D], DT.float32, kind="ExternalInput").ap()
    g = nc.dram_tensor("g", [C], DT.float32, kind="ExternalInput").ap()
    out = nc.dram_tensor("out", [D, C], DT.float32, kind="ExternalOutput").ap()
    with tile.TileContext(nc) as tc:
        _ffn(tc, xT, w1, b1, w2, b2, g, out, C)
    nc.finalize()
    return nc


def get_nc(C=C_DEFAULT):
    if C not in _CACHE:
        _CACHE[C] = _build(C)
    return _CACHE[C]


def route(x, Wr, br):
    """Host router in float64: top-2 expert ids + renormalized gates.

    The rank2/rank3 prob gap is >=2.8e-5 on this data, so any router
    accurate to ~1e-6 (f64 trivially is) selects the same experts as the
    f32 reference; gate values agree to ~3e-6.
    """
    xf = x.reshape(TOKENS, D).astype(np.float64)
    logits = xf @ Wr.astype(np.float64) + br.astype(np.float64)
    m = logits.max(axis=-1, keepdims=True)
    ez = np.exp(logits - m)
    probs = ez / ez.sum(axis=-1, keepdims=True)
    order = np.argsort(-probs, axis=-1, kind="stable")
    top2 = order[:, :TOP_K]
    p2 = np.take_along_axis(probs, top2, axis=1)
    gates = (p2 / p2.sum(axis=-1, keepdims=True)).astype(np.float32)
    return top2, gates


def dispatch(x, Wr, br):
    """Token->core assignment: expert e is served by cores 2e and 2e+1."""
    top2, gates = route(x, Wr, br)
    toks, gvals = [], []
    for e in range(E):
        hit = top2 == e                        # (TOKENS, 2)
        te = np.nonzero(hit.any(axis=1))[0]
        ge = (gates * hit).sum(axis=1)[te].astype(np.float32)
        n = len(te)
        half = (n + 1) // 2
        toks.extend([te[:half], te[half:]])
        gvals.extend([ge[:half], ge[half:]])
    return toks, gvals


def make_in_maps(inputs, C=C_DEFAULT):
    x = np.ascontiguousarray(np.asarray(inputs["x"], dtype=np.float32))
    Wr = np.asarray(inputs["Wr"], dtype=np.float32)
    br = np.asarray(inputs["br"], dtype=np.float32)
    W1 = np.asarray(inputs["W1"], dtype=np.float32)
    b1 = np.ascontiguousarray(np.asarray(inputs["b1"], dtype=np.float32))
    W2 = np.asarray(inputs["W2"], dtype=np.float32)
    b2 = np.ascontiguousarray(np.asarray(inputs["b2"], dtype=np.float32))

    toks, gvals = dispatch(x, Wr, br)
    needed = max(len(t) for t in toks)
    if needed > C:
        C = ((needed + NCC * 8 - 1) // (NCC * 8)) * NCC * 8  # NCC-divisible

    xTfull = x.reshape(TOKENS, D).T.astype(NP_MM)  # [D, TOKENS] contiguous
    w1b = [np.ascontiguousarray(W1[e]).astype(NP_MM) for e in range(E)]
    w2b = [np.ascontiguousarray(W2[e]).astype(NP_MM) for e in range(E)]
    in_maps = []
    for cid in range(N_CORES):
        e = cid // 2
        cnt = len(toks[cid])
        xt_c = np.zeros((D, C), dtype=NP_MM)
        xt_c[:, :cnt] = xTfull[:, toks[cid]]
        g_c = np.zeros((C,), dtype=np.float32)
        g_c[:cnt] = gvals[cid]
        in_maps.append({"xt": xt_c, "w1": w1b[e], "b1": b1[e],
                        "w2": w2b[e], "b2": b2[e], "g": g_c})
    return in_maps, toks, C


def kernel(x, Wr, br, W1, b1, W2, b2):
    inputs = {"x": x, "Wr": Wr, "br": br, "W1": W1, "b1": b1,
              "W2": W2, "b2": b2}
    in_maps, toks, C = make_in_maps(inputs)
    nc = get_nc(C)
    res = run_bass_kernel_spmd(nc, in_maps, core_ids=list(range(N_CORES)))
    outT = np.zeros((D, TOKENS), dtype=np.float32)
    for cid in range(N_CORES):
        cnt = len(toks[cid])
        outT[:, toks[cid]] += res.results[cid]["out"][:, :cnt]
    return np.ascontiguousarray(outT.T).reshape(B, S, D)
